# revision 57
# baseline (speedup 1.0000x reference)
"""DisparityFusion Trainium2 kernel (8 NeuronCores, SPMD data-parallel), v3.

Full inputs in, full output out. Sharding: core c handles batch b=c//4 and
output rows [64*(c%4), 64*(c%4)+64), with a 1-row halo computed locally.

v3 strategy vs v2 (156us):
  * ALL DMAs moved to HWDGE (nc.sync): the DVE fast-exp runs in 2-port perf
    mode which blocks GpSimd's SWDGE descriptor generation, so v2's
    gpsimd-issued DMAs were serialized against DVE work. HWDGE never
    contends with DVE.
  * exp work rebalanced Scalar:DVE ~ 1:2 (ACT is 1 elem/cyc @1.2GHz; DVE
    tensor_scalar hits a 2x perf mode), interleaved within each block so
    both engines stream continuously and the PE consumes pairs in
    availability order.
  * Block-major schedule: stage-1 blocks (dp rows 0-31, tail 64-65, 32-63)
    run across all 3 branches before moving on; stage-2 is chunked into
    two 32-output-row chunks so chunk 0 starts at ~70% of stage 1 and
    fills PE/Scalar/DVE gaps.
  * Stage-1 divides cut to 3 DVE ops per (branch, block) writing the f16
    d-map in place (no staging DMA). Tail block uses a [96,512] psum with
    sections at 0/32/64 so every engine AP stays 32-partition-aligned.
"""

import math
import sys

sys.path.insert(0, "/opt/trn_rl_repo")

from contextlib import ExitStack

import numpy as np
import ml_dtypes

import concourse.bass as bass
import concourse.bacc as bacc
import concourse.tile as tile
from concourse import mybir
from concourse import bass_utils
from concourse.ap import AP

B, D, H, W = 2, 192, 256, 512
N_CORES = 8
SLAB = 64            # output rows per core
SLABP = SLAB + 2     # slab + 1-row halo each side

# stage-1 blocks over the 66 dp rows: (r0, nr, j0, nj); j = packed B row
BLOCKS = [(0, 32, 0, 16), (32, 32, 16, 16), (64, 2, 32, 1)]

LN4 = math.log(4.0)
K_TRICK = 8.0 / math.log(2.0)            # 11.5416
C_TRICK = 8.0 * (7.0 - 2.0 - 0.04367)    # 39.6506
X_LO, X_HI = -3.25, 5.5

F32 = mybir.dt.float32
F16 = mybir.dt.float16
F32R = mybir.dt.float32r
F8 = mybir.dt.float8e4
I16 = mybir.dt.int16
FP8_NP = ml_dtypes.float8_e4m3
DR = mybir.MatmulPerfMode.DoubleRow
AOP = mybir.AluOpType

# exp ops per block: (start, n, is_dve) in block-local A rows / B packed
# rows. Scalar owns the spike rows (dp row 0 -> blk0 A/B starts; dp 64-65
# -> the whole tail block). DVE regions are clamped to [X_LO, X_HI] on the
# host: A dp rows [12, 52), B dp rows [8, 56).
A_OPS = [
    [(12, 10, 1), (0, 6, 0), (22, 10, 1), (6, 6, 0)],
    [(0, 10, 1), (20, 6, 0), (10, 10, 1), (26, 6, 0)],
    [(0, 2, 0)],
]
B_OPS = [
    [(6, 10, 1), (0, 6, 0)],
    [(0, 10, 1), (10, 6, 0)],
    [(0, 1, 0)],
]


def _pass_order(blk):
    # interleave passes by expected production order (DVE ~2x Scalar rate)
    if blk == 0:
        return ([("a", r) for r in (12, 14, 16, 18, 20)]
                + [("a", r) for r in (0, 2, 4)]
                + [("a", r) for r in (22, 24, 26, 28, 30)]
                + [("a", r) for r in (6, 8, 10)]
                + [("b", j) for j in (6, 8, 10)]
                + [("b", j) for j in (0, 2)]
                + [("b", j) for j in (12, 14)]
                + [("b", j) for j in (4,)])
    if blk == 1:
        return ([("a", r) for r in (0, 2, 4, 6, 8)]
                + [("a", r) for r in (20, 22, 24)]
                + [("a", r) for r in (10, 12, 14, 16, 18)]
                + [("a", r) for r in (26, 28, 30)]
                + [("b", j) for j in (0, 2, 4)]
                + [("b", j) for j in (10, 12)]
                + [("b", j) for j in (6, 8)]
                + [("b", j) for j in (14,)])
    return [("a", 0), ("s", 0)]


def _build_nc():
    nc = bacc.Bacc(
        "TRN2",
        target_bir_lowering=False,
        debug=False,
        enable_asserts=False,
        num_devices=N_CORES,
    )

    xas = [
        nc.dram_tensor(f"xa{i}", [128, SLABP, W], F8, kind="ExternalInput").ap()
        for i in (1, 2, 3)
    ]
    xbs = [
        nc.dram_tensor(f"xb{i}", [128, SLABP // 2, W], F8, kind="ExternalInput").ap()
        for i in (1, 2, 3)
    ]
    laP_d = nc.dram_tensor("laP", [128, 16, 2, 96], F8, kind="ExternalInput").ap()
    lbP_d = nc.dram_tensor("lbP", [128, 8, 2, 96], F8, kind="ExternalInput").ap()
    laT_d = nc.dram_tensor("laT", [128, 3, 2, 96], F8, kind="ExternalInput").ap()
    lbsT_d = nc.dram_tensor("lbsT", [128, 3, 96], F8, kind="ExternalInput").ap()
    wc_d = nc.dram_tensor("wc", [108, 108], F16, kind="ExternalInput").ap()
    wsA_d = nc.dram_tensor("wsA", [108, 8, 32], F32R, kind="ExternalInput").ap()
    whA_d = nc.dram_tensor("whA", [108, 8, 32], F16, kind="ExternalInput").ap()
    bv_d = nc.dram_tensor("bv", [108, 1], F32, kind="ExternalInput").ap()
    z1_d = nc.dram_tensor("z1", [1, 1], F16, kind="ExternalInput").ap()
    out_d = nc.dram_tensor("out", [SLAB, W], F32, kind="ExternalOutput").ap()

    with tile.TileContext(nc) as tc, ExitStack() as ctx:
        consts = ctx.enter_context(tc.tile_pool(name="consts", bufs=1))
        dpads = ctx.enter_context(tc.tile_pool(name="dpads", bufs=1))

        # const tiles (DMAs emitted later, interleaved with the first x
        # loads so the first exp ops can start ASAP)
        laP = consts.tile([128, 16, 2, 96], F8, tag="laP")
        lbP = consts.tile([128, 8, 2, 96], F8, tag="lbP")
        laT = consts.tile([128, 3, 2, 96], F8, tag="laT")
        lbsT = consts.tile([128, 3, 96], F8, tag="lbsT")
        wc = consts.tile([108, 108], F16, tag="wc")
        wsA = consts.tile([108, 8, 32], F32R, tag="wsA")
        whA = consts.tile([108, 8, 32], F16, tag="whA")
        bv = consts.tile([108, 1], F32, tag="bv")
        eps32 = consts.tile([32, 1], F32, tag="eps32")
        nc.vector.memset(eps32[:], 1e-8)
        nln4 = consts.tile([128, 1], F32, tag="nln4")
        nc.vector.memset(nln4[:], -LN4)
        warm3 = consts.tile([32, 3], F32, tag="warm3")
        nc.vector.memset(warm3[:], 0.25)
        warm = consts.tile([32, 3], F32, tag="warm")
        nc.scalar.activation(warm[:], warm3[:], mybir.ActivationFunctionType.Exp)

        dps = []
        for i in range(3):
            dp = dpads.tile([SLABP, W + 2], F16, tag=f"dp{i}")
            nc.vector.memset(dp[:, 0:1], 0.0)
            nc.vector.memset(dp[:, W + 1 : W + 2], 0.0)
            dps.append(dp)

        xa_p = ctx.enter_context(tc.tile_pool(name="xa", bufs=4))
        xb_p = ctx.enter_context(tc.tile_pool(name="xb", bufs=3))
        es_p = ctx.enter_context(tc.tile_pool(name="es", bufs=5))
        ev_p = ctx.enter_context(tc.tile_pool(name="ev", bufs=4))
        ps_p = ctx.enter_context(tc.tile_pool(name="ps1", bufs=2, space="PSUM"))
        dv_p = ctx.enter_context(tc.tile_pool(name="div", bufs=1))
        im_p = ctx.enter_context(tc.tile_pool(name="im", bufs=2))
        im2_p = ctx.enter_context(tc.tile_pool(name="im2", bufs=2))
        aff_p = ctx.enter_context(tc.tile_pool(name="aff", bufs=3))
        prod_p = ctx.enter_context(tc.tile_pool(name="prod", bufs=3))
        pc_p = ctx.enter_context(tc.tile_pool(name="pc", bufs=2, space="PSUM"))
        acc_p = ctx.enter_context(tc.tile_pool(name="acc", bufs=1,
                                               space="PSUM"))
        fin_p = ctx.enter_context(tc.tile_pool(name="fin", bufs=1))

        # ---- input DMAs, issued up front in processing order, split at
        # the Scalar/DVE boundary so the first exp ops start early ----
        # (blk, br) -> [(tile_row0, n), ...] pieces for A and B chunks
        A_SPLIT = [[(0, 12), (12, 20)], [(0, 20), (20, 12)], [(0, 2)]]
        B_SPLIT = [[(0, 6), (6, 10)], [(0, 10), (10, 6)], [(0, 1)]]
        xAt = {}
        xBt = {}

        def x_dmas(blk, br, a_split=None, b_split=None):
            r0, nr, j0, nj = BLOCKS[blk]
            ta = xa_p.tile([128, 32, W], F8, tag="xa")
            tb = xb_p.tile([128, 16, W], F8, tag="xb")
            xAt[(br, blk)] = ta
            xBt[(br, blk)] = tb
            for (s, n) in a_split or A_SPLIT[blk]:
                nc.sync.dma_start(ta[:, s : s + n],
                                  xas[br][:, r0 + s : r0 + s + n, :])
            for (s, n) in b_split or B_SPLIT[blk]:
                nc.sync.dma_start(tb[:, s : s + n],
                                  xbs[br][:, j0 + s : j0 + s + n, :])

        # first branch: finest split so both exp engines start ASAP
        ta0 = xa_p.tile([128, 32, W], F8, tag="xa")
        tb0 = xb_p.tile([128, 16, W], F8, tag="xb")
        xAt[(0, 0)] = ta0
        xBt[(0, 0)] = tb0
        nc.sync.dma_start(ta0[:, 0:12], xas[0][:, 0:12, :])
        nc.sync.dma_start(ta0[:, 12:22], xas[0][:, 12:22, :])
        nc.sync.dma_start(tb0[:, 0:6], xbs[0][:, 0:6, :])
        nc.sync.dma_start(ta0[:, 22:32], xas[0][:, 22:32, :])
        nc.sync.dma_start(laP[:], laP_d[:])
        nc.sync.dma_start(lbP[:], lbP_d[:])
        nc.sync.dma_start(laT[:], laT_d[:])
        nc.sync.dma_start(lbsT[:], lbsT_d[:])
        nc.sync.dma_start(tb0[:, 6:16], xbs[0][:, 6:16, :])
        x_dmas(0, 1)
        x_dmas(0, 2)
        nc.sync.dma_start(wc[:], wc_d[:])
        nc.sync.dma_start(wsA[:], wsA_d[:])
        nc.sync.dma_start(whA[:], whA_d[:])
        nc.sync.dma_start(bv[:], bv_d[:])
        for br in range(3):
            x_dmas(1, br)
        for br in range(3):
            x_dmas(2, br)

        def emit_stage1(br, blk):
            r0, nr, j0, nj = BLOCKS[blk]
            xA = xAt[(br, blk)]
            xB = xBt[(br, blk)]
            ea = {}
            eb = {}

            def emit_exp(ops, xt, table):
                for (s, n, is_dve) in ops:
                    if is_dve:
                        ei = ev_p.tile([128, 10, W], I16, tag="ev")
                        nc.vector.tensor_scalar(
                            ei[:, 0:n], xt[:, s : s + n],
                            K_TRICK, C_TRICK, AOP.mult, AOP.add,
                        )
                        ev = ei[:].bitcast(F8).rearrange(
                            "p n (w c) -> p n w c", c=2
                        )
                        table[s] = (n, ev[:, 0:n, :, 0])
                    else:
                        es = es_p.tile([128, 6, W], F8, tag="es")
                        nc.scalar.activation(
                            es[:, 0:n], xt[:, s : s + n],
                            mybir.ActivationFunctionType.Exp,
                            bias=nln4[:],
                        )
                        table[s] = (n, es[:, 0:n])

            emit_exp(A_OPS[blk], xA, ea)
            emit_exp(B_OPS[blk], xB, eb)

            def _lookup(table, idx, m=2):
                for s0 in sorted(table, reverse=True):
                    if s0 <= idx:
                        n, ap = table[s0]
                        assert idx - s0 + m <= n, (idx, s0, n, m)
                        return ap[:, idx - s0 : idx - s0 + m, :]
                raise AssertionError(idx)

            ps = ps_p.tile([96, W], F32, tag="ps")
            order = _pass_order(blk)
            n_mm = len(order)
            for pi, (kind, loc) in enumerate(order):
                st = pi == 0
                sp = pi == n_mm - 1
                tab = laP if kind == "a" else lbP
                nc.tensor.matmul(
                    ps[:], tab[:, loc // 2],
                    _lookup(ea if kind == "a" else eb, loc),
                    start=st, stop=sp, perf_mode=DR,
                )

            def divide():
                rec = dv_p.tile([32, W], F32, tag="rec")
                nc.vector.reciprocal_approx_fast(rec[0:nr], ps[0:nr])
                t1 = dv_p.tile([32, W], F16, tag="t1")
                nc.vector.tensor_mul(t1[0:nr], ps[32 : 32 + nr], rec[0:nr])
                t2 = dv_p.tile([32, W], F16, tag="t2")
                nc.vector.tensor_mul(t2[0:nr], ps[64 : 64 + nr], rec[0:nr])
                nc.vector.tensor_add(dps[br][r0 : r0 + nr, 1 : W + 1],
                                     t1[0:nr], t2[0:nr])

            return divide

        def emit_tail():
            # tail block (dp rows 64-65): all 3 branches accumulate into one
            # [96, W] psum; (s0|s1a|s1b) rows (2*br, 2*br+1) per section.
            ps = ps_p.tile([96, W], F32, tag="ps")
            eas = []
            ebs = []
            for br in range(3):
                ea = {}
                eb = {}
                xA = xAt[(br, 2)]
                xB = xBt[(br, 2)]

                es = es_p.tile([128, 6, W], F8, tag="es")
                nc.scalar.activation(
                    es[:, 0:2], xA[:, 0:2],
                    mybir.ActivationFunctionType.Exp, bias=nln4[:],
                )
                eas.append(es)
                es2 = es_p.tile([128, 6, W], F8, tag="es")
                nc.scalar.activation(
                    es2[:, 0:1], xB[:, 0:1],
                    mybir.ActivationFunctionType.Exp, bias=nln4[:],
                )
                ebs.append(es2)
            for br in range(3):
                nc.tensor.matmul(
                    ps[:], laT[:, br], eas[br][:, 0:2],
                    start=(br == 0), stop=False, perf_mode=DR,
                )
                nc.tensor.matmul(
                    ps[:], lbsT[:, br], ebs[br][:, 0, :],
                    start=False, stop=(br == 2),
                )
            rec = dv_p.tile([32, W], F32, tag="rec")
            nc.vector.reciprocal_approx_fast(rec[0:6], ps[0:6])
            t1 = dv_p.tile([32, W], F16, tag="t1")
            nc.vector.tensor_mul(t1[0:6], ps[32:38], rec[0:6])
            t2 = dv_p.tile([32, W], F16, tag="t2")
            nc.vector.tensor_mul(t2[0:6], ps[64:70], rec[0:6])
            dt = dv_p.tile([32, W], F16, tag="dt")
            nc.vector.tensor_add(dt[0:6], t1[0:6], t2[0:6])
            for br in range(3):
                nc.sync.dma_start(dps[br][64:66, 1 : W + 1],
                                  dt[2 * br : 2 * br + 2])

        def emit_block_col(blk, hook=None):
            pend = None
            pend_br = None
            for br in range(3):
                div = emit_stage1(br, blk)
                if pend is not None:
                    pend()
                    if hook:
                        hook(pend_br)
                pend = div
                pend_br = br
            pend()
            if hook:
                hook(pend_br)

        im_tiles = []
        im2_tiles = []
        for k in range(2):
            t = im_p.tile([108, 8, W], F16, tag="imk")
            nc.sync.dma_start(t[0:1, 0:1, 0:1], z1_d[:, :])
            im_tiles.append(t)
            t2 = im2_p.tile([36, 8, W + 2], F16, tag="im2")
            nc.sync.dma_start(t2[0:1, 0:1, 0:1], z1_d[:, :])
            im2_tiles.append(t2)

        def im2_dp(k, br):
            # staging tile partition 12*br + 4*dy + q holds dp rows
            # 32k+dy+8q+n at full padded width; one DMA per (branch, dy)
            for dy in range(3):
                p = 12 * br + 4 * dy
                nc.sync.dma_start(
                    im2_tiles[k][p : p + 4, :, :],
                    dps[br][32 * k + dy : 32 * k + dy + 32, :],
                )

        def imk_build(k):
            # dx-major im_k layout: p = 36*dx + 12*br + 4*dy + q
            for dx in range(3):
                nc.sync.dma_start(
                    im_tiles[k][36 * dx : 36 * dx + 36, :, :],
                    im2_tiles[k][:, :, dx : dx + W],
                )

        accs = {}
        pcss = {}

        def stage2_convs(k):
            im_k = im_tiles[k]
            pssk = acc_p.tile([32, W], F32, tag=f"pss{k}")
            psak = acc_p.tile([32, W], F32, tag=f"psa{k}")
            accs[k] = (pssk, psak)
            pcs = []
            for n in range(8):
                pc = pc_p.tile([108, W], F32, tag="pc")
                nc.tensor.matmul(pc[:], wc[:], im_k[:, n, :],
                                 start=True, stop=True)
                pcs.append(pc)
            pcss[k] = pcs

        def stage2_accs(k):
            im_k = im_tiles[k]
            pssk, psak = accs[k]
            for n in range(8):
                aff = aff_p.tile([108, W], F16, tag="aff")
                nc.scalar.activation(
                    aff[:], pcss[k][n][:], mybir.ActivationFunctionType.Relu,
                    bias=bv[:],
                )
                nc.tensor.matmul(
                    pssk[:], whA[:, n], aff[:],
                    start=(n == 0), stop=(n == 7),
                )
                prod = prod_p.tile([108, W], F32R, tag="prod")
                nc.vector.tensor_mul(prod[:], aff[:], im_k[:, n, :])
                nc.tensor.matmul(
                    psak[:], wsA[:, n], prod[:],
                    start=(n == 0), stop=(n == 7),
                )

        def emit_finale(k):
            pssk, psak = accs[k]
            den = fin_p.tile([32, W], F32, tag="den")
            nc.scalar.activation(
                den[:], pssk[:],
                mybir.ActivationFunctionType.Identity, bias=eps32[:],
            )
            rec2 = fin_p.tile([32, W], F32, tag="rec2")
            nc.vector.reciprocal_approx_fast(rec2[:], den[:])
            oc = fin_p.tile([32, W], F32, tag="oc")
            nc.vector.tensor_mul(oc[:], psak[:], rec2[:])
            out_vk = out_d[32 * k : 32 * k + 32, :].rearrange(
                "(q n) w -> n q w", q=4
            )
            nc.sync.dma_start(out_vk, oc[:])

        # ---------------- schedule ----------------
        emit_block_col(0)
        emit_block_col(1, hook=lambda br: im2_dp(0, br))
        imk_build(0)
        emit_tail()
        for br in range(3):
            im2_dp(1, br)
        imk_build(1)
        stage2_convs(0)
        stage2_accs(0)
        stage2_convs(1)
        emit_finale(0)
        stage2_accs(1)
        emit_finale(1)

    nc.compile()
    return nc


_NC_CACHE = None


def _get_nc():
    global _NC_CACHE
    if _NC_CACHE is None:
        _NC_CACHE = _build_nc()
    return _NC_CACHE


def _host_consts(W1, g1, b1, W2, g2, b2, W3, g3, b3):
    # Stage-1 DoubleRow stationaries. Column order for an A pair (r, r+1):
    # psum partitions: s0 at 2rl+i, s1a at 32+, s1b at 64+ (16*(d>>4), d&15).
    dh16_a = (16 * (np.arange(128) >> 4)).astype(np.float32)
    dl_a = (np.arange(128) & 15).astype(np.float32)
    laP = np.zeros((128, 16, 2, 96), np.float32)
    for rl in range(16):
        for i in range(2):
            laP[:, rl, i, 2 * rl + i] = 1.0
            laP[:, rl, i, 32 + 2 * rl + i] = dh16_a
            laP[:, rl, i, 64 + 2 * rl + i] = dl_a
    # B packed: partition p -> d = 128 + (p % 64); p<64 row 2j, p>=64 row 2j+1
    db = 128 + (np.arange(128) % 64)
    dh16_b = (16 * (db >> 4)).astype(np.float32)
    dl_b = (db & 15).astype(np.float32)
    lo = np.arange(128) < 64
    hi = ~lo
    lbP = np.zeros((128, 8, 2, 96), np.float32)
    for jl in range(8):
        for i in range(2):
            for half, m in ((0, lo), (1, hi)):
                c = 4 * jl + 2 * i + half
                lbP[m, jl, i, c] = 1.0
                lbP[m, jl, i, 32 + c] = dh16_b[m]
                lbP[m, jl, i, 64 + c] = dl_b[m]
    # tail (dp rows 64-65): all 3 branches share one psum; branch br owns
    # section rows (2*br, 2*br+1)
    laT = np.zeros((128, 3, 2, 96), np.float32)
    lbsT = np.zeros((128, 3, 96), np.float32)
    for br in range(3):
        for i in range(2):
            laT[:, br, i, 2 * br + i] = 1.0
            laT[:, br, i, 32 + 2 * br + i] = dh16_a
            laT[:, br, i, 64 + 2 * br + i] = dl_a
        for half, m in ((0, lo), (1, hi)):
            lbsT[m, br, 2 * br + half] = 1.0
            lbsT[m, br, 32 + 2 * br + half] = dh16_b[m]
            lbsT[m, br, 64 + 2 * br + half] = dl_b[m]

    # Stage-2: k/m space p = 36*dx + 12*br + 4*dy + q (dx-major, matching
    # the dx-shifted im_k build), tap/channel t = 3*dy + dx, q = 8-row group
    def pidx(br, t, q):
        return 36 * (t % 3) + 12 * br + 4 * (t // 3) + q

    Ws = [W1, W2, W3]
    gs = [g1, g2, g3]
    bs = [b1, b2, b3]
    wc = np.zeros((108, 108), np.float32)
    wsA = np.zeros((108, 8, 32), np.float32)
    bv = np.zeros((108, 1), np.float32)
    for br in range(3):
        wflat = Ws[br].reshape(9, 9)  # [c, tap]
        for c in range(9):
            for tap in range(9):
                for q in range(4):
                    wc[pidx(br, tap, q), pidx(br, c, q)] = (
                        wflat[c, tap] * gs[br][c]
                    )
        for c in range(9):
            for q in range(4):
                for n in range(8):
                    wsA[pidx(br, c, q), n, 4 * n + q] = 1.0
                bv[pidx(br, c, q), 0] = bs[br][c]
    f8 = lambda a: a.astype(FP8_NP)
    return f8(laP), f8(lbP), f8(laT), f8(lbsT), wc, wsA, bv


def prepare_in_maps(out_1, out_2, out_3, W1, g1, b1, W2, g2, b2, W3, g3, b3):
    xs_full = [np.asarray(a, np.float32) for a in (out_1, out_2, out_3)]
    laP, lbP, laT, lbsT, wc, wsA, bv = _host_consts(
        *[np.asarray(a, np.float32) for a in (W1, g1, b1, W2, g2, b2, W3, g3, b3)]
    )

    spike = np.full((D, 1, 1), -15.0, np.float32)
    spike[0] = 5.5

    in_maps = []
    for c in range(N_CORES):
        b = c // 4
        h0 = SLAB * (c % 4)
        lo, hi = max(0, h0 - 1), min(H, h0 + SLAB + 1)

        im = {"laP": laP, "lbP": lbP, "laT": laT, "lbsT": lbsT,
              "wc": wc.astype(np.float16), "wsA": wsA,
              "whA": wsA.astype(np.float16), "bv": bv,
              "z1": np.zeros((1, 1), np.float16)}
        for i, xf in enumerate(xs_full):
            shard = np.empty((D, SLABP, W), np.float32)
            shard[:, lo - (h0 - 1) : hi - (h0 - 1), :] = xf[b, :, lo:hi, :]
            if h0 == 0:
                shard[:, 0:1, :] = spike
            if h0 + SLAB == H:
                shard[:, SLABP - 1 :, :] = spike
            # clamp the DVE fast-exp regions (dp rows [12, 52) for both the
            # A chunk and the packed B chunk)
            np.clip(shard[:, 12:52, :], X_LO, X_HI, out=shard[:, 12:52, :])
            np.minimum(shard, X_HI, out=shard)
            im[f"xa{i + 1}"] = shard[0:128].astype(FP8_NP)
            cb = shard[128:192].reshape(64, SLABP // 2, 2, W)
            im[f"xb{i + 1}"] = np.ascontiguousarray(
                np.concatenate([cb[:, :, 0, :], cb[:, :, 1, :]], axis=0)
            ).astype(FP8_NP)
        in_maps.append(im)
    return in_maps


def gather(results):
    out = np.zeros((B, H, W), np.float32)
    for c in range(N_CORES):
        b = c // 4
        h0 = SLAB * (c % 4)
        out[b, h0 : h0 + SLAB, :] = results[c]["out"]
    return out


def kernel(**inputs):
    in_maps = prepare_in_maps(**inputs)
    res = bass_utils.run_bass_kernel_spmd(
        _get_nc(), in_maps, core_ids=list(range(N_CORES))
    )
    return gather(res.results)


# revision 59
# speedup vs baseline: 1.1793x; 1.1793x over previous
"""DisparityFusion Trainium2 kernel (8 NeuronCores, SPMD data-parallel), v3.

Full inputs in, full output out. Sharding: core c handles batch b=c//4 and
output rows [64*(c%4), 64*(c%4)+64), with a 1-row halo computed locally.

v3 strategy vs v2 (156us):
  * ALL DMAs moved to HWDGE (nc.sync): the DVE fast-exp runs in 2-port perf
    mode which blocks GpSimd's SWDGE descriptor generation, so v2's
    gpsimd-issued DMAs were serialized against DVE work. HWDGE never
    contends with DVE.
  * exp work rebalanced Scalar:DVE ~ 1:2 (ACT is 1 elem/cyc @1.2GHz; DVE
    tensor_scalar hits a 2x perf mode), interleaved within each block so
    both engines stream continuously and the PE consumes pairs in
    availability order.
  * Block-major schedule: stage-1 blocks (dp rows 0-31, tail 64-65, 32-63)
    run across all 3 branches before moving on; stage-2 is chunked into
    two 32-output-row chunks so chunk 0 starts at ~70% of stage 1 and
    fills PE/Scalar/DVE gaps.
  * Stage-1 divides cut to 3 DVE ops per (branch, block) writing the f16
    d-map in place (no staging DMA). Tail block uses a [96,512] psum with
    sections at 0/32/64 so every engine AP stays 32-partition-aligned.
"""

import math
import sys

sys.path.insert(0, "/opt/trn_rl_repo")

from contextlib import ExitStack

import numpy as np
import ml_dtypes

import concourse.bass as bass
import concourse.bacc as bacc
import concourse.tile as tile
from concourse import mybir
from concourse import bass_utils
from concourse.ap import AP

B, D, H, W = 2, 192, 256, 512
N_CORES = 8
SLAB = 64            # output rows per core
SLABP = SLAB + 2     # slab + 1-row halo each side

# stage-1 blocks over the 66 dp rows: (r0, nr, j0, nj); j = packed B row
BLOCKS = [(0, 32, 0, 16), (32, 32, 16, 16), (64, 2, 32, 1)]

LN4 = math.log(4.0)
K_TRICK = 8.0 / math.log(2.0)            # 11.5416
C_TRICK = 8.0 * (7.0 - 2.0 - 0.04367)    # 39.6506
X_LO, X_HI = -3.25, 5.5

F32 = mybir.dt.float32
F16 = mybir.dt.float16
F32R = mybir.dt.float32r
F8 = mybir.dt.float8e4
I16 = mybir.dt.int16
FP8_NP = ml_dtypes.float8_e4m3
DR = mybir.MatmulPerfMode.DoubleRow
AOP = mybir.AluOpType

# exp ops per block: (start, n, is_dve) in block-local A rows / B packed
# rows. Scalar owns the spike rows (dp row 0 -> blk0 A/B starts; dp 64-65
# -> the whole tail block). DVE regions are clamped to [X_LO, X_HI] on the
# host: A dp rows [12, 52), B dp rows [8, 56).
A_OPS = [
    [(12, 10, 1), (0, 6, 0), (22, 10, 1), (6, 6, 0)],
    [(0, 10, 1), (20, 6, 0), (10, 10, 1), (26, 6, 0)],
    [(0, 2, 0)],
]
B_OPS = [
    [(6, 10, 1), (0, 6, 0)],
    [(0, 10, 1), (10, 6, 0)],
    [(0, 1, 0)],
]


def _pass_order(blk):
    # interleave passes by expected production order (DVE ~2x Scalar rate)
    if blk == 0:
        return ([("a", r) for r in (12, 14, 16, 18, 20)]
                + [("a", r) for r in (0, 2, 4)]
                + [("a", r) for r in (22, 24, 26, 28, 30)]
                + [("a", r) for r in (6, 8, 10)]
                + [("b", j) for j in (6, 8, 10)]
                + [("b", j) for j in (0, 2)]
                + [("b", j) for j in (12, 14)]
                + [("b", j) for j in (4,)])
    if blk == 1:
        return ([("a", r) for r in (0, 2, 4, 6, 8)]
                + [("a", r) for r in (20, 22, 24)]
                + [("a", r) for r in (10, 12, 14, 16, 18)]
                + [("a", r) for r in (26, 28, 30)]
                + [("b", j) for j in (0, 2, 4)]
                + [("b", j) for j in (10, 12)]
                + [("b", j) for j in (6, 8)]
                + [("b", j) for j in (14,)])
    return [("a", 0), ("s", 0)]


def _build_nc():
    nc = bacc.Bacc(
        "TRN2",
        target_bir_lowering=False,
        debug=False,
        enable_asserts=False,
        num_devices=N_CORES,
    )

    xas = [
        nc.dram_tensor(f"xa{i}", [128, SLABP, W], F8, kind="ExternalInput").ap()
        for i in (1, 2, 3)
    ]
    xbs = [
        nc.dram_tensor(f"xb{i}", [128, SLABP // 2, W], F8, kind="ExternalInput").ap()
        for i in (1, 2, 3)
    ]
    laP_d = nc.dram_tensor("laP", [128, 16, 2, 96], F8, kind="ExternalInput").ap()
    lbP_d = nc.dram_tensor("lbP", [128, 8, 2, 96], F8, kind="ExternalInput").ap()
    laT_d = nc.dram_tensor("laT", [128, 3, 2, 96], F8, kind="ExternalInput").ap()
    lbsT_d = nc.dram_tensor("lbsT", [128, 3, 96], F8, kind="ExternalInput").ap()
    wc_d = nc.dram_tensor("wc", [108, 108], F16, kind="ExternalInput").ap()
    wsA_d = nc.dram_tensor("wsA", [108, 8, 32], F32R, kind="ExternalInput").ap()
    whA_d = nc.dram_tensor("whA", [108, 8, 32], F16, kind="ExternalInput").ap()
    bv_d = nc.dram_tensor("bv", [108, 1], F32, kind="ExternalInput").ap()
    z1_d = nc.dram_tensor("z1", [1, 1], F16, kind="ExternalInput").ap()
    out_d = nc.dram_tensor("out", [SLAB, W], F32, kind="ExternalOutput").ap()

    with tile.TileContext(nc) as tc, ExitStack() as ctx:
        consts = ctx.enter_context(tc.tile_pool(name="consts", bufs=1))
        dpads = ctx.enter_context(tc.tile_pool(name="dpads", bufs=1))

        # const tiles (DMAs emitted later, interleaved with the first x
        # loads so the first exp ops can start ASAP)
        laP = consts.tile([128, 16, 2, 96], F8, tag="laP")
        lbP = consts.tile([128, 8, 2, 96], F8, tag="lbP")
        laT = consts.tile([128, 3, 2, 96], F8, tag="laT")
        lbsT = consts.tile([128, 3, 96], F8, tag="lbsT")
        wc = consts.tile([108, 108], F16, tag="wc")
        wsA = consts.tile([108, 8, 32], F32R, tag="wsA")
        whA = consts.tile([108, 8, 32], F16, tag="whA")
        bv = consts.tile([108, 1], F32, tag="bv")
        eps32 = consts.tile([32, 1], F32, tag="eps32")
        nc.vector.memset(eps32[:], 1e-8)
        nln4 = consts.tile([128, 1], F32, tag="nln4")
        nc.vector.memset(nln4[:], -LN4)
        warm3 = consts.tile([32, 3], F32, tag="warm3")
        nc.vector.memset(warm3[:], 0.25)
        warm = consts.tile([32, 3], F32, tag="warm")
        nc.scalar.activation(warm[:], warm3[:], mybir.ActivationFunctionType.Exp)

        dps = []
        for i in range(3):
            dp = dpads.tile([SLABP, W + 2], F16, tag=f"dp{i}")
            nc.vector.memset(dp[:, 0:1], 0.0)
            nc.vector.memset(dp[:, W + 1 : W + 2], 0.0)
            dps.append(dp)

        xa_p = ctx.enter_context(tc.tile_pool(name="xa", bufs=4))
        xb_p = ctx.enter_context(tc.tile_pool(name="xb", bufs=4))
        es_p = ctx.enter_context(tc.tile_pool(name="es", bufs=6))
        ev_p = ctx.enter_context(tc.tile_pool(name="ev", bufs=4))
        ps_p = ctx.enter_context(tc.tile_pool(name="ps1", bufs=2, space="PSUM"))
        dv_p = ctx.enter_context(tc.tile_pool(name="div", bufs=2))
        im_p = ctx.enter_context(tc.tile_pool(name="im", bufs=2))
        aff_p = ctx.enter_context(tc.tile_pool(name="aff", bufs=3))
        prod_p = ctx.enter_context(tc.tile_pool(name="prod", bufs=3))
        pc_p = ctx.enter_context(tc.tile_pool(name="pc", bufs=2, space="PSUM"))
        acc_p = ctx.enter_context(tc.tile_pool(name="acc", bufs=1,
                                               space="PSUM"))
        fin_p = ctx.enter_context(tc.tile_pool(name="fin", bufs=1))

        # ---- input DMAs, issued up front in processing order, split at
        # the Scalar/DVE boundary so the first exp ops start early ----
        # (blk, br) -> [(tile_row0, n), ...] pieces for A and B chunks
        A_SPLIT = [[(0, 12), (12, 20)], [(0, 20), (20, 12)], [(0, 2)]]
        B_SPLIT = [[(0, 6), (6, 10)], [(0, 10), (10, 6)], [(0, 1)]]
        xAt = {}
        xBt = {}

        def x_dmas(blk, br, a_split=None, b_split=None):
            r0, nr, j0, nj = BLOCKS[blk]
            ta = xa_p.tile([128, 32, W], F8, tag="xa")
            tb = xb_p.tile([128, 16, W], F8, tag="xb")
            xAt[(br, blk)] = ta
            xBt[(br, blk)] = tb
            for (s, n) in a_split or A_SPLIT[blk]:
                nc.sync.dma_start(ta[:, s : s + n],
                                  xas[br][:, r0 + s : r0 + s + n, :])
            for (s, n) in b_split or B_SPLIT[blk]:
                nc.sync.dma_start(tb[:, s : s + n],
                                  xbs[br][:, j0 + s : j0 + s + n, :])

        # first branch: finest split so both exp engines start ASAP
        ta0 = xa_p.tile([128, 32, W], F8, tag="xa")
        tb0 = xb_p.tile([128, 16, W], F8, tag="xb")
        xAt[(0, 0)] = ta0
        xBt[(0, 0)] = tb0
        nc.sync.dma_start(ta0[:, 0:12], xas[0][:, 0:12, :])
        nc.sync.dma_start(ta0[:, 12:22], xas[0][:, 12:22, :])
        nc.sync.dma_start(tb0[:, 0:6], xbs[0][:, 0:6, :])
        nc.sync.dma_start(ta0[:, 22:32], xas[0][:, 22:32, :])
        nc.sync.dma_start(laP[:], laP_d[:])
        nc.sync.dma_start(lbP[:], lbP_d[:])
        nc.sync.dma_start(laT[:], laT_d[:])
        nc.sync.dma_start(lbsT[:], lbsT_d[:])
        nc.sync.dma_start(tb0[:, 6:16], xbs[0][:, 6:16, :])
        x_dmas(0, 1)
        x_dmas(0, 2)
        nc.sync.dma_start(wc[:], wc_d[:])
        nc.sync.dma_start(wsA[:], wsA_d[:])
        nc.sync.dma_start(whA[:], whA_d[:])
        nc.sync.dma_start(bv[:], bv_d[:])
        for br in range(3):
            x_dmas(1, br)
        for br in range(3):
            x_dmas(2, br)

        def emit_stage1(br, blk):
            r0, nr, j0, nj = BLOCKS[blk]
            xA = xAt[(br, blk)]
            xB = xBt[(br, blk)]
            ea = {}
            eb = {}

            def emit_exp(ops, xt, table):
                for (s, n, is_dve) in ops:
                    if is_dve:
                        ei = ev_p.tile([128, 10, W], I16, tag="ev")
                        nc.vector.tensor_scalar(
                            ei[:, 0:n], xt[:, s : s + n],
                            K_TRICK, C_TRICK, AOP.mult, AOP.add,
                        )
                        ev = ei[:].bitcast(F8).rearrange(
                            "p n (w c) -> p n w c", c=2
                        )
                        table[s] = (n, ev[:, 0:n, :, 0])
                    else:
                        es = es_p.tile([128, 6, W], F8, tag="es")
                        nc.scalar.activation(
                            es[:, 0:n], xt[:, s : s + n],
                            mybir.ActivationFunctionType.Exp,
                            bias=nln4[:],
                        )
                        table[s] = (n, es[:, 0:n])

            emit_exp(A_OPS[blk], xA, ea)
            emit_exp(B_OPS[blk], xB, eb)

            def _lookup(table, idx, m=2):
                for s0 in sorted(table, reverse=True):
                    if s0 <= idx:
                        n, ap = table[s0]
                        assert idx - s0 + m <= n, (idx, s0, n, m)
                        return ap[:, idx - s0 : idx - s0 + m, :]
                raise AssertionError(idx)

            ps = ps_p.tile([96, W], F32, tag="ps")
            order = _pass_order(blk)
            n_mm = len(order)
            for pi, (kind, loc) in enumerate(order):
                st = pi == 0
                sp = pi == n_mm - 1
                tab = laP if kind == "a" else lbP
                nc.tensor.matmul(
                    ps[:], tab[:, loc // 2],
                    _lookup(ea if kind == "a" else eb, loc),
                    start=st, stop=sp, perf_mode=DR,
                )

            def divide():
                rec = dv_p.tile([32, W], F32, tag="rec")
                nc.vector.reciprocal_approx_fast(rec[0:nr], ps[0:nr])
                t1 = dv_p.tile([32, W], F16, tag="t1")
                nc.vector.tensor_mul(t1[0:nr], ps[32 : 32 + nr], rec[0:nr])
                t2 = dv_p.tile([32, W], F16, tag="t2")
                nc.vector.tensor_mul(t2[0:nr], ps[64 : 64 + nr], rec[0:nr])
                nc.vector.tensor_add(dps[br][r0 : r0 + nr, 1 : W + 1],
                                     t1[0:nr], t2[0:nr])

            return divide

        def emit_tail():
            # tail block (dp rows 64-65): all 3 branches accumulate into one
            # [96, W] psum; (s0|s1a|s1b) rows (2*br, 2*br+1) per section.
            ps = ps_p.tile([96, W], F32, tag="ps")
            eas = []
            ebs = []
            for br in range(3):
                ea = {}
                eb = {}
                xA = xAt[(br, 2)]
                xB = xBt[(br, 2)]

                es = es_p.tile([128, 6, W], F8, tag="es")
                nc.scalar.activation(
                    es[:, 0:2], xA[:, 0:2],
                    mybir.ActivationFunctionType.Exp, bias=nln4[:],
                )
                eas.append(es)
                es2 = es_p.tile([128, 6, W], F8, tag="es")
                nc.scalar.activation(
                    es2[:, 0:1], xB[:, 0:1],
                    mybir.ActivationFunctionType.Exp, bias=nln4[:],
                )
                ebs.append(es2)
            for br in range(3):
                nc.tensor.matmul(
                    ps[:], laT[:, br], eas[br][:, 0:2],
                    start=(br == 0), stop=False, perf_mode=DR,
                )
                nc.tensor.matmul(
                    ps[:], lbsT[:, br], ebs[br][:, 0, :],
                    start=False, stop=(br == 2),
                )
            rec = dv_p.tile([32, W], F32, tag="rec")
            nc.vector.reciprocal_approx_fast(rec[0:6], ps[0:6])
            t1 = dv_p.tile([32, W], F16, tag="t1")
            nc.vector.tensor_mul(t1[0:6], ps[32:38], rec[0:6])
            t2 = dv_p.tile([32, W], F16, tag="t2")
            nc.vector.tensor_mul(t2[0:6], ps[64:70], rec[0:6])
            dt = dv_p.tile([32, W], F16, tag="dt")
            nc.vector.tensor_add(dt[0:6], t1[0:6], t2[0:6])
            for br in range(3):
                nc.sync.dma_start(dps[br][64:66, 1 : W + 1],
                                  dt[2 * br : 2 * br + 2])

        def emit_block_col(blk, hook=None):
            pend = None
            pend_br = None
            for br in range(3):
                div = emit_stage1(br, blk)
                if pend is not None:
                    pend()
                    if hook:
                        hook(pend_br)
                pend = div
                pend_br = br
            pend()
            if hook:
                hook(pend_br)

        im_tiles = []
        for k in range(2):
            t = im_p.tile([108, 8, W], F16, tag="imk")
            nc.sync.dma_start(t[0:1, 0:1, 0:1], z1_d[:, :])
            im_tiles.append(t)

        def im2col(k, br, dys):
            im_k = im_tiles[k]
            for dy in dys:
                for dx in range(3):
                    p = (9 * br + 3 * dy + dx) * 4
                    nc.sync.dma_start(
                        im_k[p : p + 4, :, :],
                        dps[br][32 * k + dy : 32 * k + dy + 32, dx : dx + W],
                    )

        accs = {}
        pss0 = acc_p.tile([32, W], F32, tag="pss0")
        pss1 = acc_p.tile([32, W], F32, tag="pss1")
        psa0 = acc_p.tile([32, W], F32, tag="psa0")
        psa1 = acc_p.tile([32, W], F32, tag="psa1")
        accs[0] = (pss0, psa0)
        accs[1] = (pss1, psa1)

        def emit_stage2(k):
            im_k = im_tiles[k]
            pssk, psak = accs[k]
            for n in range(8):
                pc = pc_p.tile([108, W], F32, tag="pc")
                nc.tensor.matmul(pc[:], wc[:], im_k[:, n, :],
                                 start=True, stop=True)
                aff = aff_p.tile([108, W], F16, tag="aff")
                nc.scalar.activation(
                    aff[:], pc[:], mybir.ActivationFunctionType.Relu,
                    bias=bv[:],
                )
                nc.tensor.matmul(
                    pssk[:], whA[:, n], aff[:],
                    start=(n == 0), stop=(n == 7),
                )
                prod = prod_p.tile([108, W], F32R, tag="prod")
                nc.vector.tensor_mul(prod[:], aff[:], im_k[:, n, :])
                nc.tensor.matmul(
                    psak[:], wsA[:, n], prod[:],
                    start=(n == 0), stop=(n == 7),
                )

        def emit_finale(k):
            pssk, psak = accs[k]
            den = fin_p.tile([32, W], F32, tag="den")
            nc.scalar.activation(
                den[:], pssk[:],
                mybir.ActivationFunctionType.Identity, bias=eps32[:],
            )
            rec2 = fin_p.tile([32, W], F32, tag="rec2")
            nc.vector.reciprocal_approx_fast(rec2[:], den[:])
            oc = fin_p.tile([32, W], F32, tag="oc")
            nc.vector.tensor_mul(oc[:], psak[:], rec2[:])
            out_vk = out_d[32 * k : 32 * k + 32, :].rearrange(
                "(q n) w -> n q w", q=4
            )
            nc.sync.dma_start(out_vk, oc[:])

        # ---------------- schedule ----------------
        emit_block_col(0, hook=lambda br: im2col(0, br, [0]))
        emit_block_col(1, hook=lambda br: (im2col(0, br, [1, 2]),
                                           im2col(1, br, [0])))
        emit_stage2(0)
        emit_tail()
        for br in range(3):
            im2col(1, br, [1, 2])
        emit_finale(0)
        emit_stage2(1)
        emit_finale(1)

    nc.compile()
    return nc


_NC_CACHE = None


def _get_nc():
    global _NC_CACHE
    if _NC_CACHE is None:
        _NC_CACHE = _build_nc()
    return _NC_CACHE


def _host_consts(W1, g1, b1, W2, g2, b2, W3, g3, b3):
    # Stage-1 DoubleRow stationaries. Column order for an A pair (r, r+1):
    # psum partitions: s0 at 2rl+i, s1a at 32+, s1b at 64+ (16*(d>>4), d&15).
    dh16_a = (16 * (np.arange(128) >> 4)).astype(np.float32)
    dl_a = (np.arange(128) & 15).astype(np.float32)
    laP = np.zeros((128, 16, 2, 96), np.float32)
    for rl in range(16):
        for i in range(2):
            laP[:, rl, i, 2 * rl + i] = 1.0
            laP[:, rl, i, 32 + 2 * rl + i] = dh16_a
            laP[:, rl, i, 64 + 2 * rl + i] = dl_a
    # B packed: partition p -> d = 128 + (p % 64); p<64 row 2j, p>=64 row 2j+1
    db = 128 + (np.arange(128) % 64)
    dh16_b = (16 * (db >> 4)).astype(np.float32)
    dl_b = (db & 15).astype(np.float32)
    lo = np.arange(128) < 64
    hi = ~lo
    lbP = np.zeros((128, 8, 2, 96), np.float32)
    for jl in range(8):
        for i in range(2):
            for half, m in ((0, lo), (1, hi)):
                c = 4 * jl + 2 * i + half
                lbP[m, jl, i, c] = 1.0
                lbP[m, jl, i, 32 + c] = dh16_b[m]
                lbP[m, jl, i, 64 + c] = dl_b[m]
    # tail (dp rows 64-65): all 3 branches share one psum; branch br owns
    # section rows (2*br, 2*br+1)
    laT = np.zeros((128, 3, 2, 96), np.float32)
    lbsT = np.zeros((128, 3, 96), np.float32)
    for br in range(3):
        for i in range(2):
            laT[:, br, i, 2 * br + i] = 1.0
            laT[:, br, i, 32 + 2 * br + i] = dh16_a
            laT[:, br, i, 64 + 2 * br + i] = dl_a
        for half, m in ((0, lo), (1, hi)):
            lbsT[m, br, 2 * br + half] = 1.0
            lbsT[m, br, 32 + 2 * br + half] = dh16_b[m]
            lbsT[m, br, 64 + 2 * br + half] = dl_b[m]

    # Stage-2: k/m space p = (9*br + t)*4 + q, q = 8-row group of the chunk
    def pidx(br, t, q):
        return (9 * br + t) * 4 + q

    Ws = [W1, W2, W3]
    gs = [g1, g2, g3]
    bs = [b1, b2, b3]
    wc = np.zeros((108, 108), np.float32)
    wsA = np.zeros((108, 8, 32), np.float32)
    bv = np.zeros((108, 1), np.float32)
    for br in range(3):
        wflat = Ws[br].reshape(9, 9)  # [c, tap]
        for c in range(9):
            for tap in range(9):
                for q in range(4):
                    wc[pidx(br, tap, q), pidx(br, c, q)] = (
                        wflat[c, tap] * gs[br][c]
                    )
        for c in range(9):
            for q in range(4):
                for n in range(8):
                    wsA[pidx(br, c, q), n, 4 * n + q] = 1.0
                bv[pidx(br, c, q), 0] = bs[br][c]
    f8 = lambda a: a.astype(FP8_NP)
    return f8(laP), f8(lbP), f8(laT), f8(lbsT), wc, wsA, bv


def prepare_in_maps(out_1, out_2, out_3, W1, g1, b1, W2, g2, b2, W3, g3, b3):
    xs_full = [np.asarray(a, np.float32) for a in (out_1, out_2, out_3)]
    laP, lbP, laT, lbsT, wc, wsA, bv = _host_consts(
        *[np.asarray(a, np.float32) for a in (W1, g1, b1, W2, g2, b2, W3, g3, b3)]
    )

    spike = np.full((D, 1, 1), -15.0, np.float32)
    spike[0] = 5.5

    in_maps = []
    for c in range(N_CORES):
        b = c // 4
        h0 = SLAB * (c % 4)
        lo, hi = max(0, h0 - 1), min(H, h0 + SLAB + 1)

        im = {"laP": laP, "lbP": lbP, "laT": laT, "lbsT": lbsT,
              "wc": wc.astype(np.float16), "wsA": wsA,
              "whA": wsA.astype(np.float16), "bv": bv,
              "z1": np.zeros((1, 1), np.float16)}
        for i, xf in enumerate(xs_full):
            shard = np.empty((D, SLABP, W), np.float32)
            shard[:, lo - (h0 - 1) : hi - (h0 - 1), :] = xf[b, :, lo:hi, :]
            if h0 == 0:
                shard[:, 0:1, :] = spike
            if h0 + SLAB == H:
                shard[:, SLABP - 1 :, :] = spike
            # clamp the DVE fast-exp regions (dp rows [12, 52) for both the
            # A chunk and the packed B chunk)
            np.clip(shard[:, 12:52, :], X_LO, X_HI, out=shard[:, 12:52, :])
            np.minimum(shard, X_HI, out=shard)
            im[f"xa{i + 1}"] = shard[0:128].astype(FP8_NP)
            cb = shard[128:192].reshape(64, SLABP // 2, 2, W)
            im[f"xb{i + 1}"] = np.ascontiguousarray(
                np.concatenate([cb[:, :, 0, :], cb[:, :, 1, :]], axis=0)
            ).astype(FP8_NP)
        in_maps.append(im)
    return in_maps


def gather(results):
    out = np.zeros((B, H, W), np.float32)
    for c in range(N_CORES):
        b = c // 4
        h0 = SLAB * (c % 4)
        out[b, h0 : h0 + SLAB, :] = results[c]["out"]
    return out


def kernel(**inputs):
    in_maps = prepare_in_maps(**inputs)
    res = bass_utils.run_bass_kernel_spmd(
        _get_nc(), in_maps, core_ids=list(range(N_CORES))
    )
    return gather(res.results)
